# revision 16
# baseline (speedup 1.0000x reference)
"""AxialSelfAttentionModule kernel — Trainium (8 NeuronCores) + CPU tail.

Device (D-sharded, 4 d-planes/core x B=2): circ_conv3 + inorm/gelu + lp2 +
md/vm branches + vector-gate chain -> x_mod  (~89% of FLOPs, bf16 matmuls,
two tiny stats AllReduces for the global InstanceNorms).
Host (CPU jax): qkv matmul + 3 axial attentions + proj (~11% of FLOPs).

Math notes (validated vs fp32 reference):
  - rotations cancel (orthonormal R, norms restored) -> skipped.
  - pos_attn & q-side norm bias are constant along the softmax axis -> only
    the k-side bias 0.1*(kn @ vnp_w) survives.
  - conv/md1/vm1 biases cancel inside InstanceNorm -> skipped.
Channel permutation: x / md2 / vm2 outputs use (comp, group) channel order
(c_new = comp*64 + g, c_orig = 3g + comp) so 3-vector norms reduce within a
64-partition block on device; qkv_w columns are permuted to match on host.
"""

import numpy as np
from contextlib import ExitStack

B, C, D, H, W = 2, 192, 32, 32, 32
NH, HD, NV, NVC = 16, 12, 4, 64
SCALE = HD ** -0.5
S = D * H * W
NCORE = 8
DL = D // NCORE              # 4 d-planes per core
PLANE = H * W                # 1024 tokens per plane
TOKC = B * DL * PLANE        # 8192 tokens per core

_PERM = (3 * (np.arange(192) % 64) + np.arange(192) // 64).astype(np.int64)
# _PERM[c_new] = c_orig with c_new = comp*64+g

_ENGINE = {}


def _build_device():
    import concourse.bass as bass
    import concourse.mybir as mybir
    from concourse.tile import TileContext
    from concourse.vector_clock import ScopedClock

    dt = mybir.dt
    AF = mybir.ActivationFunctionType

    class SafeDrainTC(TileContext):
        # walrus rejects >1 sem wait on the final Drain: keep one, move the
        # rest onto chained SP nops before the all-engine barrier.
        def _drain_and_barrier(self, tick_clock, wait_clock):
            nc = self.nc
            drain_inst = nc.sync.drain()
            wait_clock.add_sem_waits(
                drain_inst.ins, ScopedClock({None: tick_clock.global_clock})
            )
            si = drain_inst.ins.sync_info
            if si is not None and si.on_wait and len(si.on_wait) > 1:
                extra = list(si.on_wait[1:])
                del si.on_wait[1:]
                for w in extra:
                    nop = nc.sync.nop(nofuse=True)
                    nsi = nop.ins.sync_info
                    if nsi is None:
                        nsi = mybir.SyncInfo(on_wait=[], on_update=[])
                        nop.ins.sync_info = nsi
                    nsi.on_wait.append(w)
            nc.all_engine_barrier()
            assert self.sems is not None
            popped = nc._tile_sem_poison_stack.pop()
            assert popped is self._sem_poison
            nc.clear_and_free_semaphores(list(self.sems.allocated().values()))
            nc.all_engine_barrier()

    nc = bass.Bass()
    f32, bf16 = dt.float32, dt.bfloat16

    pe = nc.declare_dram_parameter("pe", [B, C, 6 * 34 * 34], f32, isOutput=False)
    xin = nc.declare_dram_parameter("xin", [3, 64, TOKC], f32, isOutput=False)
    wconv = nc.declare_dram_parameter("wconv", [C, 27 * C], f32, isOutput=False)
    wmats = nc.declare_dram_parameter("wmats", [C, 5 * C], f32, isOutput=False)
    b192 = nc.declare_dram_parameter("b192", [C, 1], f32, isOutput=False)
    b64 = nc.declare_dram_parameter("b64", [64, 8], f32, isOutput=False)
    xmod = nc.declare_dram_parameter("xmod", [3, 64, TOKC], bf16, isOutput=True)


    core_ids = list(range(NCORE))
    PS = [(0, 128), (128, 64)]          # channel chunks (offset, size)
    NPL = B * DL                        # 8 planes per core
    X = mybir.AxisListType.X

    ctx = ExitStack()
    with SafeDrainTC(nc, num_cores=NCORE) as tc:
        wpool = ctx.enter_context(tc.tile_pool(name="wpool", bufs=1))
        spool = ctx.enter_context(tc.tile_pool(name="spool", bufs=1))
        stg = ctx.enter_context(tc.tile_pool(name="stg", bufs=1))
        wk16 = ctx.enter_context(tc.tile_pool(name="wk16", bufs=2))
        wk3 = ctx.enter_context(tc.tile_pool(name="wk3", bufs=3))
        wks = ctx.enter_context(tc.tile_pool(name="wks", bufs=1))
        psum = ctx.enter_context(tc.tile_pool(name="psum", bufs=2, space="PSUM"))
        dpool = ctx.enter_context(tc.tile_pool(name="dram", bufs=1, space="DRAM"))
        y1d = dpool.tile([C, TOKC], bf16, name="y1d")
        locald = dpool.tile([C, TOKC], bf16, name="locald")
        md1d = dpool.tile([C, TOKC], bf16, name="md1d")
        vm1d = dpool.tile([C, TOKC], bf16, name="vm1d")
        cc1i = dpool.tile([C, 4], f32, name="cc1i")
        cc1o = dpool.tile([C, 4], f32, name="cc1o", addr_space="Shared")
        cc2i = dpool.tile([C, 8], f32, name="cc2i")
        cc2o = dpool.tile([C, 8], f32, name="cc2o", addr_space="Shared")

        # ---- load weights / padded pos_emb to SBUF, convert to bf16 ----
        wcv, wmt, pep = [], [], []
        for off, p in PS:
            t = stg.tile([p, 5184], f32, tag="stage")
            nc.sync.dma_start(out=t[:], in_=wconv[off:off + p, :])
            tb = wpool.tile([p, 27 * C], bf16, tag=f"wcv{p}")
            nc.vector.tensor_copy(tb[:], t[:])
            wcv.append(tb)
            t2 = stg.tile([p, 5 * C], f32, tag="stage")
            nc.sync.dma_start(out=t2[:], in_=wmats[off:off + p, :])
            tb2 = wpool.tile([p, 5 * C], bf16, tag=f"wmt{p}")
            nc.vector.tensor_copy(tb2[:], t2[:])
            wmt.append(tb2)
            pt = wpool.tile([p, B * 6936], bf16, tag=f"pep{p}")
            for b in range(B):
                for dz in range(6):
                    ts = stg.tile([p, 1156], f32, tag="stage")
                    nc.sync.dma_start(out=ts[:], in_=pe[b, off:off + p, dz * 1156:(dz + 1) * 1156])
                    nc.vector.tensor_copy(
                        pt[:, b * 6936 + dz * 1156: b * 6936 + (dz + 1) * 1156], ts[:])
            pep.append(pt)
        bias192 = []
        for off, p in PS:
            t = spool.tile([p, 1], f32, tag=f"b192_{p}")
            nc.sync.dma_start(out=t[:], in_=b192[off:off + p, :])
            bias192.append(t)
        bias64 = spool.tile([64, 8], f32, tag="b64")
        nc.sync.dma_start(out=bias64[:], in_=b64[:, :])

        stats1 = [spool.tile([p, 4], f32, tag=f"st1_{p}", name=f"stats1_{p}") for _, p in PS]
        stats2 = [spool.tile([p, 8], f32, tag=f"st2_{p}", name=f"stats2_{p}") for _, p in PS]
        for st in stats1 + stats2:
            nc.vector.memset(st[:], 0.0)

        pviews = [pt.rearrange("p (b dz hp wp) -> p b dz hp wp", b=B, dz=6, hp=34, wp=34)
                  for pt in pep]

        def stat_acc(stats, col, ps_tile, sb_copy, p):
            red = wks.tile([p, 1], f32, tag="red")
            nc.vector.reduce_sum(red[:], ps_tile[:], axis=X)
            nc.vector.tensor_add(stats[:, col:col + 1], stats[:, col:col + 1], red[:])
            sq = wks.tile([p, PLANE], f32, tag="sqbuf")
            nc.vector.tensor_mul(sq[:], sb_copy[:], sb_copy[:])
            red2 = wks.tile([p, 1], f32, tag="red2")
            nc.vector.reduce_sum(red2[:], sq[:], axis=X)
            nc.vector.tensor_add(stats[:, col + 1:col + 2], stats[:, col + 1:col + 2], red2[:])

        def norm_factors(g, col, p, tagp):
            r = spool.tile([p, 1], f32, tag=f"r{tagp}")
            nbt = spool.tile([p, 1], f32, tag=f"n{tagp}")
            mean = wks.tile([p, 1], f32, tag="mean")
            nc.vector.tensor_scalar_mul(mean[:], g[:, col:col + 1], 1.0 / S)
            msq = wks.tile([p, 1], f32, tag="msq")
            nc.vector.tensor_mul(msq[:], mean[:], mean[:])
            var = wks.tile([p, 1], f32, tag="var")
            nc.vector.tensor_scalar_mul(var[:], g[:, col + 1:col + 2], 1.0 / S)
            nc.vector.tensor_sub(var[:], var[:], msq[:])
            nc.vector.tensor_scalar_add(var[:], var[:], 1e-5)
            sd = wks.tile([p, 1], f32, tag="sd")
            nc.scalar.activation(sd[:], var[:], AF.Sqrt)
            nc.vector.reciprocal(r[:], sd[:])
            nc.vector.tensor_mul(nbt[:], mean[:], r[:])
            nc.vector.tensor_scalar_mul(nbt[:], nbt[:], -1.0)
            return r, nbt

        def mm1x1(pp, wsl, rhss):
            for h0 in (0, 512):
                for kc in range(2):
                    nc.tensor.matmul(pp[:, h0:h0 + 512], wsl[kc], rhss[kc][:, h0:h0 + 512],
                                     start=(kc == 0), stop=(kc == 1))

        # ---- Phase A: conv -> y1 planes + stats ----
        for ip in range(NPL):
            b, dl = ip // DL, ip % DL
            for mc, (moff, mp) in enumerate(PS):
                pp = psum.tile([mp, PLANE], f32, tag=f"p{mp}")
                for h0 in (0, 16):
                    first = True
                    for kc in range(2):
                        for t in range(27):
                            tz, ty, tx = t // 9, (t // 3) % 3, t % 3
                            rhs = pviews[kc][:, b, dl + tz, ty + h0:ty + h0 + 16, tx:tx + 32]
                            lhsT = wcv[kc][:, t * C + moff: t * C + moff + mp]
                            nc.tensor.matmul(pp[:, h0 * 32:(h0 + 16) * 32], lhsT, rhs,
                                             start=first, stop=(kc == 1 and t == 26))
                            first = False
                yb = wk16.tile([mp, PLANE], bf16, tag=f"yb{mp}")
                nc.scalar.activation(yb[:], pp[:], AF.Copy)
                nc.sync.dma_start(out=y1d[moff:moff + mp, ip * PLANE:(ip + 1) * PLANE], in_=yb[:])
                stat_acc(stats1[mc], 2 * b, pp, yb, mp)

        # ---- Phase B: AllReduce stats1 -> (scale, bias) per chunk per b ----
        for c, (off, p) in enumerate(PS):
            nc.sync.dma_start(out=cc1i[off:off + p, :], in_=stats1[c][:])
        nc.gpsimd.collective_compute(
            "AllReduce", mybir.AluOpType.add, replica_groups=[core_ids],
            ins=[cc1i.opt()], outs=[cc1o.opt()])
        fac1 = []
        for c, (off, p) in enumerate(PS):
            g = spool.tile([p, 4], f32, tag=f"g1_{p}")
            nc.sync.dma_start(out=g[:], in_=cc1o[off:off + p, :])
            fac1.append([norm_factors(g, 2 * b, p, f"1_{p}_{b}") for b in range(B)])

        # ---- Phase C: g1=gelu(inorm(y1)); local; md1/vm1 + stats ----
        for ip in range(NPL):
            b, dl = ip // DL, ip % DL
            g1 = []
            for c, (off, p) in enumerate(PS):
                yt = wk16.tile([p, PLANE], bf16, tag=f"yt{p}")
                nc.sync.dma_start(out=yt[:], in_=y1d[off:off + p, ip * PLANE:(ip + 1) * PLANE])
                gt = wk16.tile([p, PLANE], bf16, tag=f"gt{p}")
                nc.scalar.activation(gt[:], yt[:], AF.Gelu,
                                     bias=fac1[c][b][1][:], scale=fac1[c][b][0][:])
                g1.append(gt)
            lt = []
            for mc, (moff, mp) in enumerate(PS):
                pp = psum.tile([mp, PLANE], f32, tag=f"p{mp}")
                mm1x1(pp, [wmt[kc][:, moff:moff + mp] for kc in range(2)], g1)
                loct = wk16.tile([mp, PLANE], bf16, tag=f"loc{mp}")
                nc.vector.tensor_scalar_add(loct[:], pp[:], bias192[mc][:])
                nc.sync.dma_start(out=locald[moff:moff + mp, ip * PLANE:(ip + 1) * PLANE],
                                  in_=loct[:])
                lt.append(loct)
            for widx, scol, dram in ((1, 0, md1d), (2, 4, vm1d)):
                for mc, (moff, mp) in enumerate(PS):
                    pp = psum.tile([mp, PLANE], f32, tag=f"p{mp}")
                    mm1x1(pp, [wmt[kc][:, widx * C + moff:widx * C + moff + mp] for kc in range(2)], lt)
                    ob = wk16.tile([mp, PLANE], bf16, tag=f"ob{mp}")
                    nc.scalar.activation(ob[:], pp[:], AF.Copy)
                    nc.sync.dma_start(out=dram[moff:moff + mp, ip * PLANE:(ip + 1) * PLANE],
                                      in_=ob[:])
                    stat_acc(stats2[mc], scol + 2 * b, pp, ob, mp)

        # ---- Phase D: AllReduce stats2 -> factors for md1, vm1 ----
        for c, (off, p) in enumerate(PS):
            nc.sync.dma_start(out=cc2i[off:off + p, :], in_=stats2[c][:])
        nc.gpsimd.collective_compute(
            "AllReduce", mybir.AluOpType.add, replica_groups=[core_ids],
            ins=[cc2i.opt()], outs=[cc2o.opt()])
        fac2 = []
        for c, (off, p) in enumerate(PS):
            g = spool.tile([p, 8], f32, tag=f"g2_{p}")
            nc.sync.dma_start(out=g[:], in_=cc2o[off:off + p, :])
            fac2.append([[norm_factors(g, 4 * br + 2 * b, p, f"2_{p}_{br}_{b}")
                          for b in range(B)] for br in range(2)])

        # ---- Phase E: mod/vm/x chain -> xmod ----
        for ip in range(NPL):
            b, dl = ip // DL, ip % DL
            gm, gv = [], []
            for c, (off, p) in enumerate(PS):
                mt = wk16.tile([p, PLANE], bf16, tag=f"yt{p}")
                nc.sync.dma_start(out=mt[:], in_=md1d[off:off + p, ip * PLANE:(ip + 1) * PLANE])
                gmt = wk16.tile([p, PLANE], bf16, tag=f"gt{p}")
                nc.scalar.activation(gmt[:], mt[:], AF.Gelu,
                                     bias=fac2[c][0][b][1][:], scale=fac2[c][0][b][0][:])
                gm.append(gmt)
                vt = wk16.tile([p, PLANE], bf16, tag=f"loc{p}")
                nc.sync.dma_start(out=vt[:], in_=vm1d[off:off + p, ip * PLANE:(ip + 1) * PLANE])
                gvt = wk16.tile([p, PLANE], bf16, tag=f"ob{p}")
                nc.scalar.activation(gvt[:], vt[:], AF.Gelu,
                                     bias=fac2[c][1][b][1][:], scale=fac2[c][1][b][0][:])
                gv.append(gvt)
            mod3, vm3 = [], []
            for comp in range(3):
                pp = psum.tile([64, PLANE], f32, tag="p64")
                mm1x1(pp, [wmt[kc][:, 3 * C + comp * 64:3 * C + (comp + 1) * 64] for kc in range(2)], gm)
                mo = wk3.tile([64, PLANE], f32, tag="mo3")
                nc.scalar.activation(mo[:], pp[:], AF.Sigmoid, bias=bias64[:, comp:comp + 1])
                nc.vector.tensor_scalar_add(mo[:], mo[:], 0.1)
                mod3.append(mo)
                pq = psum.tile([64, PLANE], f32, tag="p64")
                mm1x1(pq, [wmt[kc][:, 4 * C + comp * 64:4 * C + (comp + 1) * 64] for kc in range(2)], gv)
                vo = wk3.tile([64, PLANE], f32, tag="vo3")
                nc.vector.tensor_scalar_add(vo[:], pq[:], bias64[:, 3 + comp:4 + comp])
                vm3.append(vo)
            xt = []
            for comp in range(3):
                xs = wk3.tile([64, PLANE], f32, tag="xs3")
                nc.sync.dma_start(out=xs[:], in_=xin[comp, :, ip * PLANE:(ip + 1) * PLANE])
                xt.append(xs)
            sqx = wks.tile([64, PLANE], f32, tag="sqbuf")
            tmp = wks.tile([64, PLANE], f32, tag="tmpx")
            nc.vector.tensor_mul(sqx[:], xt[0][:], xt[0][:])
            nc.vector.tensor_mul(tmp[:], xt[1][:], xt[1][:])
            nc.vector.tensor_add(sqx[:], sqx[:], tmp[:])
            nc.vector.tensor_mul(tmp[:], xt[2][:], xt[2][:])
            nc.vector.tensor_add(sqx[:], sqx[:], tmp[:])
            vn = wks.tile([64, PLANE], f32, tag="vnb")
            nc.scalar.activation(vn[:], sqx[:], AF.Sqrt)
            nc.vector.tensor_mul(sqx[:], vm3[0][:], vm3[0][:])
            nc.vector.tensor_mul(tmp[:], vm3[1][:], vm3[1][:])
            nc.vector.tensor_add(sqx[:], sqx[:], tmp[:])
            nc.vector.tensor_mul(tmp[:], vm3[2][:], vm3[2][:])
            nc.vector.tensor_add(sqx[:], sqx[:], tmp[:])
            rsv = wks.tile([64, PLANE], f32, tag="rsvb")
            sdv = wks.tile([64, PLANE], f32, tag="sdvb")
            nc.vector.tensor_scalar_add(sqx[:], sqx[:], 1e-16)
            nc.scalar.activation(sdv[:], sqx[:], AF.Sqrt)
            nc.vector.reciprocal(rsv[:], sdv[:])
            gates = wks.tile([64, PLANE], f32, tag="gateb")
            nc.scalar.activation(gates[:], vn[:], AF.Sigmoid,
                                 bias=bias64[:, 7:8], scale=bias64[:, 6:7])
            t2 = wks.tile([64, PLANE], f32, tag="t2b")
            nc.vector.tensor_mul(t2[:], gates[:], vn[:])
            nc.vector.tensor_mul(t2[:], t2[:], rsv[:])
            nc.vector.tensor_scalar_mul(t2[:], t2[:], 0.1)
            for comp in range(3):
                a = wks.tile([64, PLANE], f32, tag="sqbuf")
                nc.vector.tensor_mul(a[:], xt[comp][:], mod3[comp][:])
                nc.vector.tensor_mul(tmp[:], t2[:], vm3[comp][:])
                ob = wk16.tile([64, PLANE], bf16, tag="obx")
                nc.vector.tensor_add(ob[:], a[:], tmp[:])
                nc.sync.dma_start(out=xmod[comp, :, ip * PLANE:(ip + 1) * PLANE], in_=ob[:])
        ctx.close()
    # walrus allows only one sem wait per instruction in this toolchain:
    # hoist extra waits onto same-engine nops inserted just before.
    ctr = 0
    for fn in nc.m.functions:
        for bb in fn.blocks:
            new = []
            for inst in bb.instructions:
                si = inst.sync_info
                if si is not None and si.on_wait and len(si.on_wait) > 1:
                    extras = list(si.on_wait[:-1])
                    del si.on_wait[:-1]
                    for w in extras:
                        ctr += 1
                        nop = mybir.InstNoOp(
                            name=f"I-wsplit-{ctr}", engine=inst.engine,
                            opcode="NoOp", ins=[], outs=[],
                            sync_info=mybir.SyncInfo(on_wait=[w], on_update=[]),
                            debug=inst.debug, bass_nofuse=True)
                        new.append(nop)
                new.append(inst)
            bb.instructions[:] = new
    return nc


def _host_prep(a):
    f = np.float32
    pe_pad = np.pad(a["pos_emb"], ((0, 0), (0, 0), (1, 1), (1, 1), (1, 1)),
                    mode="wrap").astype(f)
    wconv = np.ascontiguousarray(
        a["lp1_w"].transpose(1, 2, 3, 4, 0).reshape(C, 27 * C)).astype(f)
    wmats = np.concatenate(
        [a["lp2_w"].T, a["md1_w"].T, a["vm1_w"].T,
         a["md2_w"].T[:, _PERM], a["vm2_w"].T[:, _PERM]], axis=1).astype(f)
    wmats = np.ascontiguousarray(wmats)
    b192 = a["lp2_b"].reshape(C, 1).astype(f)
    b64 = np.concatenate(
        [a["md2_b"][_PERM].reshape(3, 64).T, a["vm2_b"][_PERM].reshape(3, 64).T,
         a["vng_w"].reshape(64, 1), a["vng_b"].reshape(64, 1)], axis=1).astype(f)
    b64 = np.ascontiguousarray(b64)
    xp = a["x"].reshape(B, NVC, 3, D, H, W).transpose(2, 1, 0, 3, 4, 5)
    in_maps = []
    for i in range(NCORE):
        d0 = i * DL
        pe_slab = np.ascontiguousarray(
            pe_pad[:, :, d0:d0 + DL + 2, :, :]).reshape(B, C, 6 * 34 * 34)
        x_slab = np.ascontiguousarray(
            xp[:, :, :, d0:d0 + DL, :, :]).reshape(3, 64, TOKC)
        in_maps.append({"pe": pe_slab, "xin": x_slab, "wconv": wconv,
                        "wmats": wmats, "b192": b192, "b64": b64})
    return in_maps


_TAIL = None


def _get_tail():
    global _TAIL
    if _TAIL is not None:
        return _TAIL
    import jax
    import jax.numpy as jnp
    cpu = jax.devices("cpu")[0]

    def tail(xm, qkv_wp, qkv_b, vnp_w, proj_w, proj_b):
        qkv = jnp.einsum("oc,bcs->bso", qkv_wp, xm) + qkv_b[None, None, :]
        qkv = qkv.reshape(B, D, H, W, 3, NH, HD)
        q = qkv[..., 0, :, :]
        k = qkv[..., 1, :, :]
        v = qkv[..., 2, :, :]
        kn = jnp.sqrt((k.reshape(B, D, H, W, NH, NV, 3) ** 2).sum(-1))
        kbias = jnp.asarray(0.1, jnp.float32) * jnp.einsum(
            "bdhwnv,v->bdhwn", kn, vnp_w[0])

        def axial(perm, sh1, sh2, axis):
            qa = jnp.transpose(q, perm)
            ka = jnp.transpose(k, perm)
            va = jnp.transpose(v, perm)
            kb = jnp.transpose(kbias, perm[:4] + (4,))
            L = qa.shape[3]
            bd = B * sh1 * sh2
            qa = qa.reshape(bd, L, NH, HD).transpose(0, 2, 1, 3)
            ka = ka.reshape(bd, L, NH, HD).transpose(0, 2, 1, 3)
            va = va.reshape(bd, L, NH, HD).transpose(0, 2, 1, 3)
            kb = kb.reshape(bd, L, NH).transpose(0, 2, 1)
            logits = jnp.matmul(qa, ka.transpose(0, 1, 3, 2)) * jnp.asarray(
                SCALE, jnp.float32)
            logits = logits + kb[:, :, None, :]
            p = jax.nn.softmax(logits, axis=-1)
            o = jnp.matmul(p, va)
            o = o.transpose(0, 2, 1, 3).reshape(B, sh1, sh2, L, C)
            if axis == "depth":
                o = o.transpose(0, 3, 1, 2, 4)
            elif axis == "height":
                o = o.transpose(0, 1, 3, 2, 4)
            return o.reshape(B, S, C)

        out = (axial((0, 2, 3, 1, 4, 5), H, W, "depth")
               + axial((0, 1, 3, 2, 4, 5), D, W, "height")
               + axial((0, 1, 2, 3, 4, 5), D, H, "width"))
        out = jnp.einsum("oc,bsc->bos", proj_w, out) + proj_b[None, :, None]
        return out.reshape(B, C, D, H, W).astype(jnp.float32)

    try:
        _TAIL = jax.jit(tail, device=cpu)
    except TypeError:
        base = jax.jit(tail)

        def _TAIL(*args):
            with jax.default_device(cpu):
                return base(*[jax.device_put(v, cpu) for v in args])
    return _TAIL


def kernel(x, pos_emb, qkv_w, qkv_b, lp1_w, lp1_b, lp2_w, lp2_b,
           vm1_w, vm1_b, vm2_w, vm2_b, md1_w, md1_b, md2_w, md2_b,
           pa_w, pa_b, vng_w, vng_b, vnp_w, vnp_b,
           R6_d, R6_h, R6_w, proj_w, proj_b):
    import jax
    try:
        jax.config.update("jax_compilation_cache_dir", "/tmp/axial_jax_cache_v1")
        jax.config.update("jax_persistent_cache_min_compile_time_secs", 0.0)
        jax.config.update("jax_persistent_cache_min_entry_size_bytes", 0)
    except Exception:
        pass
    from concourse.bass_utils import run_bass_kernel_spmd

    a = {k: np.asarray(v, np.float32) for k, v in dict(
        x=x, pos_emb=pos_emb, qkv_w=qkv_w, qkv_b=qkv_b, lp1_w=lp1_w,
        lp2_w=lp2_w, lp2_b=lp2_b, vm1_w=vm1_w, vm2_w=vm2_w, vm2_b=vm2_b,
        md1_w=md1_w, md2_w=md2_w, md2_b=md2_b, vng_w=vng_w, vng_b=vng_b,
        vnp_w=vnp_w, proj_w=proj_w, proj_b=proj_b).items()}

    nc = _ENGINE.get("nc")
    if nc is None:
        nc = _build_device()
        _ENGINE["nc"] = nc

    in_maps = _host_prep(a)
    res = run_bass_kernel_spmd(nc, in_maps, core_ids=list(range(NCORE)))

    xm = np.empty((3, 64, B, D, H, W), np.float32)
    for i in range(NCORE):
        xm[:, :, :, i * DL:(i + 1) * DL] = res.results[i]["xmod"].astype(
            np.float32).reshape(3, 64, B, DL, H, W)
    xm_bcs = np.ascontiguousarray(xm.transpose(2, 0, 1, 3, 4, 5)).reshape(B, C, S)

    tail = _get_tail()
    out = tail(xm_bcs, a["qkv_w"][:, _PERM], a["qkv_b"], a["vnp_w"],
               a["proj_w"], a["proj_b"])
    return np.asarray(out)
